# revision 4
# baseline (speedup 1.0000x reference)
"""Trainium2 Bass kernel for nn_Attention_layer_1580547966657.

Reference computation (B=8, S=2048, E=H=1024):
    q,k,v = x@W{q,k,v}.T + b;  scores = q@k.T/sqrt(H), query-row masked;
    att = softmax(scores) @ v;  out = att @ Wn.T  -> (B, S, 1)
    out = softmax(out, axis=-1)                   -> (B, S, 1)

The final softmax is over an axis of size 1, so the output is exactly
1.0 everywhere for any finite inputs: softmax of a single element is
exp(x-x)/exp(x-x) = 1. All upstream values stay finite for any
realistically-scaled finite inputs (masking uses a large-negative
constant, not -inf, and the row softmax over S is max-shifted), so the
whole attention pipeline algebraically cancels out of the output.

The kernel therefore only has to materialize ones((B,S,1), f32):
B=8 is sharded one batch row per core (data-parallel over batch, per the
sharding hint). Each core's program is a single DMA of an embedded
8 KiB constant (ones, loaded to HBM at NEFF load time) to its output
shard, plus the DMA-completion semaphore wait. TimelineSim: ~3.2 us/core
(NRT launch overhead dominates; the payload DMA is <1 us).
"""

import numpy as np

import concourse.bass as bass
import concourse.mybir as mybir
from concourse.bass_utils import run_bass_kernel_spmd

B, S = 8, 2048
P, N = 128, S // 128  # per-core output viewed as [128 partitions, 16 elems]

_cache = {}


def _build():
    nc = bass.Bass()
    out = nc.dram_tensor("out", (P, N), mybir.dt.float32, kind="ExternalOutput")
    ones = nc.inline_tensor(np.ones((P, N), np.float32), name="ones_const")
    s_d = nc.alloc_semaphore("s_d")
    nc.sync.dma_start(out[:], ones[:]).then_inc(s_d, 16)
    nc.sync.wait_ge(s_d, 16)
    return nc


def kernel(
    x=None, mask=None, Wq=None, bq=None, Wk=None, bk=None, Wv=None, bv=None,
    Wn=None, **_ignored,
):
    if "nc" not in _cache:
        _cache["nc"] = _build()
    res = run_bass_kernel_spmd(
        _cache["nc"], [{} for _ in range(B)], core_ids=list(range(B))
    )
    return np.stack([r["out"].reshape(S, 1) for r in res.results])


# revision 5
# speedup vs baseline: 1.4368x; 1.4368x over previous
"""Trainium2 Bass kernel for nn_Attention_layer_1580547966657.

Reference computation (B=8, S=2048, E=H=1024):
    q,k,v = x@W{q,k,v}.T + b;  scores = q@k.T/sqrt(H), query-row masked;
    att = softmax(scores) @ v;  out = att @ Wn.T  -> (B, S, 1)
    out = softmax(out, axis=-1)                   -> (B, S, 1)

The final softmax is over an axis of size 1, so the output is exactly
1.0 everywhere for any finite inputs: softmax of a single element is
exp(x-x)/exp(x-x) = 1. All upstream values stay finite for any
realistically-scaled finite inputs (masking uses a large-negative
constant, not -inf, and the row softmax over S is max-shifted), so the
whole attention pipeline algebraically cancels out of the output.

The kernel therefore only has to materialize ones((B,S,1), f32):
B=8 is sharded one batch row per core (data-parallel over batch, per the
sharding hint). Each core's program is a single DMA of a NEFF-embedded
8 KiB ones constant (placed in HBM at model-load time) to its output
shard, plus the DMA-completion semaphore wait.

The Bass() constructor preamble (per-engine register inits, builtin
const-AP memsets, and a 5-engine entry barrier) exists to protect
features this kernel never uses, so it is stripped from the emitted
block, leaving 3 instructions: the DMA-table dummy call, the DMACopy,
and the completion wait. TimelineSim: 2248 ns/core, fully accounted for
by the single DMA's hardware pipeline (HWDGE generation 625 ns +
DGE->DMA start 650 ns + 16x512B descriptor transfer ~23 ns + semaphore
propagation 900 ns + sequencer decode) — the floor for any kernel that
writes its output from the device.
"""

import numpy as np

import concourse.bass as bass
import concourse.mybir as mybir
from concourse.bass_utils import run_bass_kernel_spmd

B, S = 8, 2048
P, N = 128, S // 128  # per-core output viewed as [128 partitions, 16 elems]

_cache = {}


def _build():
    nc = bass.Bass(enable_partition_id=False, monotonic_sem_count=0)
    out = nc.dram_tensor("out", (P, N), mybir.dt.float32, kind="ExternalOutput")
    ones = nc.inline_tensor(np.ones((P, N), np.float32), name="ones_const")
    s_d = nc.alloc_semaphore("s_d")
    keep = set()
    keep.add(nc.sync.dma_start(out[:], ones[:]).then_inc(s_d, 16).ins.name)
    keep.add(nc.sync.wait_ge(s_d, 16).ins.name)
    # Drop the constructor preamble (engine reg inits, const-AP memsets,
    # entry barrier): nothing in this kernel reads const APs or runs on the
    # other engines, and NRT zeroes user semaphores in its own per-exec
    # preamble. Keep the InstCall: call_to_physical_memlocs references it
    # for the DMA table.
    bb = nc.m.functions[0].blocks[0]
    bb.instructions = [
        i for i in bb.instructions
        if i.name in keep or type(i).__name__ == "InstCall"
    ]
    return nc


def kernel(
    x=None, mask=None, Wq=None, bq=None, Wk=None, bk=None, Wv=None, bv=None,
    Wn=None, **_ignored,
):
    if "nc" not in _cache:
        _cache["nc"] = _build()
    res = run_bass_kernel_spmd(
        _cache["nc"], [{} for _ in range(B)], core_ids=list(range(B))
    )
    return np.stack([r["out"].reshape(S, 1) for r in res.results])
